# revision 16
# baseline (speedup 1.0000x reference)
"""GNN message passing (gather + segment-sum) on 8 Trainium2 cores.

out[n, :] = sum over edges e with dst_e == n of x[src_e, :]

Strategy: shard edges by destination-node range (6250 nodes per core), so each
core owns a disjoint slice of the output and no cross-core reduction is
needed. x is cast to bf16 on host and packed as node PAIRS (one 256-byte
gather element = nodes 2k and 2k+1), so the indexed DMA gather needs no
f32->bf16 cast on device and all pair indices fit int16. Edges are grouped by
(dst tile, src parity); each 128-edge chunk multiplies a one-hot S matrix
(built on the vector engine from an iota compare) against the gathered bf16
pair rows, accumulating into a per-tile PSUM bank.

Gather descriptor generation on the GPSIMD Q7 cores is the bottleneck
(~8.5ns/index on one SWDGE queue). The chunk stream is cut into ~27-chunk
windows issued round-robin over all 4 SWDGE queues (4 independent Q7 core
pairs), with a tiny primer call first and back-pressure waits placed 7
windows behind so the pool engine's pipeline never drains — descriptor
generation then runs 4-wide continuously. Per-queue A/B completion
semaphores alternate every 4 windows so every wait threshold equals its
maximum possible value (no cross-DMA increment aliasing).
"""

import numpy as np

from concourse import bass, library_config, mybir
from concourse.bass_utils import run_bass_kernel_spmd

N_NODES = 50000
D = 64
N_CORES = 8
NODES_PER_CORE = N_NODES // N_CORES  # 6250
P = 128
N_TILES = (NODES_PER_CORE + P - 1) // P  # 49
N_PAIRS = N_NODES // 2  # 25000 (< 32768, fits int16)
PSUM_BANKS = 8
N_MSGS = 12  # msgs banks; gather pacing waits reference 3 rounds back
N_S = 4  # S-matrix banks

_f32 = mybir.dt.float32
_i16 = mybir.dt.int16
_bf16 = mybir.dt.bfloat16


def _round_up(a, b):
    return (a + b - 1) // b * b


N_ABS = 3  # absorber tiles (46..48) soak up edge-count variance
CAP_TARGET = 2040  # capped-tile edge budget: sides stay under 1024 after split


def _balance_tiles(dst_in_core):
    """Assign each of the core's 6250 dst nodes a (tile, slot). Tiles
    0..45 are balanced to ~CAP_TARGET edges each (so each side of the
    parity split stays under 1024 = 8 chunks); the highest-degree nodes are
    pre-routed to the 3 absorber tiles until the remaining total fits the
    capped budget. Returns slot_of_node [6250]."""
    deg = np.bincount(dst_in_core, minlength=NODES_PER_CORE)
    order = np.argsort(-deg, kind="stable")
    n_capped = N_TILES - N_ABS
    cap = np.full(N_TILES, P, dtype=np.int64)
    cap[N_TILES - 1] = NODES_PER_CORE - (N_TILES - 1) * P  # 106
    load = np.zeros(N_TILES, dtype=np.int64)
    fill = np.zeros(N_TILES, dtype=np.int64)
    slot = np.empty(NODES_PER_CORE, dtype=np.int64)

    overflow = int(deg.sum()) - n_capped * CAP_TARGET
    absorbers = list(range(n_capped, N_TILES))
    i = 0
    while overflow > 0 and i < len(order):
        n = order[i]
        open_a = [t for t in absorbers if fill[t] < cap[t]]
        if not open_a:
            break
        t = min(open_a, key=lambda t: load[t])
        slot[n] = t * P + fill[t]
        fill[t] += 1
        load[t] += deg[n]
        overflow -= int(deg[n])
        i += 1
    for n in order[i:]:
        open_t = np.nonzero(fill[:n_capped] < cap[:n_capped])[0]
        if len(open_t):
            t = open_t[np.argmin(load[open_t])]
        else:
            t = min(
                (t for t in absorbers if fill[t] < cap[t]),
                key=lambda t: load[t],
            )
        slot[n] = t * P + fill[t]
        fill[t] += 1
        load[t] += deg[n]
    return slot


def _balance_sides(src, tile_of_edge):
    """Choose a side (0/1) per source node so that every tile's edges split
    near-evenly between sides, with exactly N_PAIRS nodes per side. Batched
    greedy on the running per-tile imbalance."""
    order = np.argsort(src, kind="stable")
    s_sorted = src[order]
    t_sorted = tile_of_edge[order]
    starts = np.searchsorted(s_sorted, np.arange(N_NODES + 1))
    Dt = np.zeros(N_TILES, dtype=np.int64)
    side = np.zeros(N_NODES, dtype=np.int8)
    quota = [N_PAIRS, N_PAIRS]
    for n in range(N_NODES):
        ts = t_sorted[starts[n] : starts[n + 1]]
        if len(ts) == 0:
            s = 0 if quota[0] >= quota[1] else 1
            if quota[s] == 0:
                s = 1 - s
        else:
            s = 0 if Dt[ts].sum() < 0 else 1
            if quota[s] == 0:
                s = 1 - s
            np.add.at(Dt, ts, 1 if s == 0 else -1)
        side[n] = s
        quota[s] -= 1
    return side


def prepare(x, edge_index):
    """Host-side sharding: shard edges by dst-node range (output ownership),
    then per core (1) permute the core's dst nodes across tiles to equalize
    per-tile edge counts, (2) split source nodes into two balanced sides and
    pair one L node with one R node per 256-byte gather element, so the
    per-(tile, side) group capacities — and with them the padded gather index
    count — are minimized. Builds per-core x-pair / index / relative-dst maps."""
    import ml_dtypes

    dst = np.asarray(edge_index[0], dtype=np.int64)
    src = np.asarray(edge_index[1], dtype=np.int64)

    core = dst // NODES_PER_CORE
    dst_in_core = (dst - core * NODES_PER_CORE).astype(np.int64)

    n_groups = N_TILES * 2  # group id = tile*2 + side
    counts = np.zeros((N_CORES, n_groups), dtype=np.int64)
    per_core = []
    xp_maps = []
    slot_maps = []
    x_bf = x.astype(ml_dtypes.bfloat16)
    for k in range(N_CORES):
        sel = np.nonzero(core == k)[0]
        d_k = dst_in_core[sel]
        s_k = src[sel]
        slot = _balance_tiles(d_k)  # [6250] -> tile*128 + m
        tile_e = slot[d_k] // P
        m_e = slot[d_k] % P
        side = _balance_sides(s_k, tile_e)  # [50000] -> 0/1
        L = np.nonzero(side == 0)[0]
        R = np.nonzero(side == 1)[0]
        assert len(L) == N_PAIRS and len(R) == N_PAIRS
        pairpos = np.empty(N_NODES, dtype=np.int64)
        pairpos[L] = np.arange(N_PAIRS)
        pairpos[R] = np.arange(N_PAIRS)
        x_pairs = np.empty((N_PAIRS, 2 * D), dtype=ml_dtypes.bfloat16)
        x_pairs[:, :D] = x_bf[L]
        x_pairs[:, D:] = x_bf[R]
        xp_maps.append(x_pairs)
        slot_maps.append(slot)

        b_e = side[s_k].astype(np.int64)
        idx_e = pairpos[s_k].astype(np.int16)
        g = (tile_e * 2 + b_e).astype(np.int64)
        order = np.argsort(g, kind="stable")
        counts[k] = np.bincount(g, minlength=n_groups)
        per_core.append((idx_e[order], m_e[order], g[order]))

    # per-group 128-aligned capacity (same on all cores: SPMD layout)
    V = _round_up(np.maximum(counts.max(axis=0), 1), P).astype(np.int64)

    # stream order: (tile, parity); chunk windows cut across groups freely
    stream_off = np.zeros(n_groups, dtype=np.int64)
    off = 0
    for g in range(n_groups):
        stream_off[g] = off
        off += V[g]
    total_v = int(off)  # multiple of 128
    n_chunks = total_v // P
    idx_cols = total_v // 16

    idx_maps = []
    dstrel_maps = []

    for k in range(N_CORES):
        idx_e, m_e, g = per_core[k]
        gc = counts[k]
        starts = np.concatenate([[0], np.cumsum(gc)[:-1]])
        rank = np.arange(len(g)) - starts[g]
        pos = stream_off[g] + rank

        # pads: idx=0 (valid, gathers pair 0); killed by dstrel=-1 in S
        idx_flat = np.zeros(total_v, dtype=np.int16)
        dstrel_flat = np.full(total_v, -1.0, dtype=np.float32)
        idx_flat[pos] = idx_e
        dstrel_flat[pos] = m_e.astype(np.float32)

        # idx wrapped: element i -> partition i%16, column i//16, replicated
        # across the 8 groups of 16 partitions (required for all SWDGE queues)
        idx_wrapped = np.ascontiguousarray(
            np.tile(idx_flat.reshape(-1, 16).T, (8, 1))
        )  # [128, idx_cols]
        dstrel_cols = np.ascontiguousarray(
            dstrel_flat.reshape(-1, P).T.astype(ml_dtypes.bfloat16)
        )  # [128, n_chunks]
        idx_maps.append(idx_wrapped)
        dstrel_maps.append(dstrel_cols)

    iota = np.tile(
        np.arange(P, dtype=np.float32).astype(ml_dtypes.bfloat16), (P, 1)
    )  # [128,128]

    meta = dict(
        V=V,
        stream_off=stream_off,
        total_v=total_v,
        idx_cols=idx_cols,
        n_chunks=n_chunks,
    )
    return xp_maps, slot_maps, idx_maps, dstrel_maps, iota, meta


def build_program(meta):
    V = meta["V"]
    idx_cols = meta["idx_cols"]
    n_chunks = meta["n_chunks"]

    # global chunk list: (tile, parity) in (tile asc, parity asc) order
    chunk_tp = []
    for t in range(N_TILES):
        for b in (0, 1):
            for _ in range(int(V[t * 2 + b]) // P):
                chunk_tp.append((t, b))
    assert len(chunk_tp) == n_chunks

    tile_first_chunk = {}
    tile_last_chunk = {}
    for ci, (t, b) in enumerate(chunk_tp):
        if t not in tile_first_chunk:
            tile_first_chunk[t] = ci
        tile_last_chunk[t] = ci

    # windows: 9 rounds of 4 (window w runs on SWDGE queue w%4). Round 0
    # starts with a 1-chunk primer and stays small (fast pipeline fill),
    # round 8 is small (short drain tail), the 6 middle rounds carry the
    # bulk. The mid windows are sized per queue so every queue's TOTAL chunk
    # count is equal (the span ends when the slowest queue finishes; the
    # primer queue would otherwise finish ~7 chunks early and idle).
    first = [1, 8, 8, 8]
    # the last round is small so the final drain-lags + matmul tail are
    # short; its queues are compensated in the mid rounds by the equalizer
    last = [8, 8, 8, 4]
    q_fixed = [first[q] + 16 + last[q] for q in range(4)]
    q_target = [n_chunks // 4 + (1 if q < n_chunks % 4 else 0) for q in range(4)]
    mid_q = [q_target[q] - q_fixed[q] for q in range(4)]  # over 6 mid rounds
    assert min(mid_q) >= 6, mid_q
    mid_sizes = []
    for r in range(6):
        for q in range(4):
            mid_sizes.append(mid_q[q] // 6 + (1 if r < mid_q[q] % 6 else 0))
    win_sizes = first + [16] * 4 + mid_sizes + last
    win_start = [0]
    for s in win_sizes:
        win_start.append(win_start[-1] + s)
    assert win_start[-1] == n_chunks
    n_windows = len(win_sizes)  # 36
    max_wc = max(win_sizes)

    # chunk -> window
    chunk_win = []
    for w, s in enumerate(win_sizes):
        chunk_win.extend([w] * s)

    # per-window completed tile range (for per-window output stores)
    tiles_done_by_win = []
    td = 0
    for w in range(n_windows):
        end_chunk = win_start[w + 1]
        while td < N_TILES and tile_last_chunk[td] < end_chunk:
            td += 1
        tiles_done_by_win.append(td)

    nc = bass.Bass(num_swdge_queues=4)
    x = nc.declare_dram_parameter("x", [N_PAIRS, 2 * D], _bf16, isOutput=False)
    idx = nc.declare_dram_parameter("idx", [P, idx_cols], _i16, isOutput=False)
    dstrel = nc.declare_dram_parameter("dstrel", [P, n_chunks], _bf16, isOutput=False)
    iota = nc.declare_dram_parameter("iota", [P, P], _bf16, isOutput=False)
    y = nc.declare_dram_parameter("y", [N_TILES * P, D], _f32, isOutput=True)

    import contextlib

    ctx = contextlib.ExitStack()
    idx_sb = ctx.enter_context(nc.sbuf_tensor("idx_sb", [P, idx_cols], _i16))
    dstrel_sb = ctx.enter_context(nc.sbuf_tensor("dstrel_sb", [P, n_chunks], _bf16))
    iota_sb = ctx.enter_context(nc.sbuf_tensor("iota_sb", [P, P], _bf16))
    acc_sb = ctx.enter_context(nc.sbuf_tensor("acc_sb", [P, N_TILES * D], _f32))
    msgs_sb = [
        ctx.enter_context(nc.sbuf_tensor(f"msgs{i}", [P, max_wc * 2 * D], _bf16))
        for i in range(N_MSGS)
    ]
    s_sb = [
        ctx.enter_context(nc.sbuf_tensor(f"s{i}", [P, max_wc * P], _bf16))
        for i in range(N_S)
    ]
    psum = [
        ctx.enter_context(nc.psum_tensor(f"ps{i}", [P, D], _f32))
        for i in range(PSUM_BANKS)
    ]

    # idx loaded per round (one DMA + semaphore per round of 4 windows)
    n_rounds = n_windows // 4
    spans = []
    for r in range(n_rounds):
        spans.append((win_start[4 * r] * P, win_start[4 * r + 4] * P))

    ixl_sems = [nc.alloc_semaphore(f"ixl{r}") for r in range(n_rounds)]
    dq_flat = [nc.alloc_semaphore(f"dq{i}") for i in range(12)]
    with (
        nc.Block() as block,
        nc.semaphore("ld_sem") as ld_sem,
        nc.semaphore("s_sem") as s_sem,
        nc.semaphore("mm_sem") as mm_sem,
        nc.semaphore("cp_sem") as cp_sem,
        nc.semaphore("st_sem") as st_sem,
    ):
        def dq_sem(w):
            return dq_flat[((w // 4) % 3) * 4 + (w % 4)]

        def dq_target(w):
            return 16 * (w // 12 + 1)

        @block.sync
        def _(sync: bass.BassEngine):
            for i, (c0, c1) in enumerate(spans):
                sync.dma_start(
                    out=idx_sb[:, c0 // 16 : c1 // 16],
                    in_=idx[:, c0 // 16 : c1 // 16],
                ).then_inc(ixl_sems[i], 16)
            sync.dma_start(out=dstrel_sb[:], in_=dstrel[:]).then_inc(ld_sem, 16)
            sync.dma_start(out=iota_sb[:], in_=iota[:]).then_inc(ld_sem, 16)
            # store each window's newly-completed output tiles
            n_stores = 0
            prev_td = 0
            for w in range(n_windows):
                td = tiles_done_by_win[w]
                if td == prev_td:
                    continue
                sync.wait_ge(cp_sem, td)
                sync.dma_start(
                    out=y[prev_td * P : td * P].rearrange("(t p) f -> p t f", p=P),
                    in_=acc_sb[:, prev_td * D : td * D].rearrange(
                        "p (t f) -> p t f", f=D
                    ),
                ).then_inc(st_sem, 16)
                n_stores += 1
                prev_td = td
            sync.wait_ge(st_sem, 16 * n_stores)

        @block.gpsimd
        def _(gpsimd: bass.BassEngine):
            # One wait cluster per ROUND of 4 windows (one window per SWDGE
            # queue), then 4 back-to-back gather dispatches: any wait placed
            # between gathers executes only after the preceding gather's Q7
            # desc-gen retires, so per-window waits would serialize the
            # queues. Waits reference 3 rounds back (12 msgs banks), which
            # also caps per-queue in-flight calls at 2 and makes the dq
            # A/B/C thresholds exact — do not loosen.
            gpsimd.load_library(library_config.mlp)
            assert n_windows % 4 == 0
            for r in range(n_windows // 4):
                ws = range(4 * r, 4 * r + 4)
                gpsimd.wait_ge(ixl_sems[r], 16)
                if r >= 3:
                    # msgs bank reuse: PE consumed round r-3's windows
                    gpsimd.wait_ge(mm_sem, win_start[4 * (r - 3) + 4])
                for w in ws:
                    c0, c1 = win_start[w], win_start[w + 1]
                    wc = c1 - c0
                    cap = wc * P
                    out_view = msgs_sb[w % N_MSGS][:, : wc * 2 * D].rearrange(
                        "p (c f) -> p c f", f=2 * D
                    )
                    gpsimd.dma_gather(
                        out_ap=out_view,
                        in_ap=x[:, :],
                        idxs_ap=idx_sb[:, c0 * P // 16 : c1 * P // 16],
                        num_idxs=cap,
                        num_idxs_reg=cap,
                        elem_size=2 * D,
                        single_packet=False,
                        queue_num=w % 4,
                    ).then_inc(dq_sem(w), 16)

        @block.vector
        def _(vector: bass.BassEngine):
            vector.wait_ge(ld_sem, 32)  # dstrel + iota loaded
            for w in range(n_windows):
                c0, c1 = win_start[w], win_start[w + 1]
                wc = c1 - c0
                if w >= N_S:
                    # S bank reuse: PE consumed window w-N_S
                    vector.wait_ge(mm_sem, win_start[w - N_S + 1])
                vector.tensor_tensor(
                    out=s_sb[w % N_S][:, : wc * P].rearrange(
                        "p (c f) -> p c f", f=P
                    ),
                    in0=dstrel_sb[:, c0:c1]
                    .rearrange("p (c o) -> p c o", o=1)
                    .to_broadcast([P, wc, P]),
                    in1=iota_sb[:, :]
                    .rearrange("p (o f) -> p o f", o=1)
                    .to_broadcast([P, wc, P]),
                    op=mybir.AluOpType.is_equal,
                ).then_inc(s_sem, 1)

        @block.tensor
        def _(tensor: bass.BassEngine):
            for ci, (t, b) in enumerate(chunk_tp):
                w = chunk_win[ci]
                lc = ci - win_start[w]
                if ci == win_start[w]:
                    tensor.wait_ge(s_sem, w + 1)
                    tensor.wait_ge(dq_sem(w), dq_target(w))
                start = ci == tile_first_chunk[t]
                stop = ci == tile_last_chunk[t]
                if start and t >= PSUM_BANKS:
                    tensor.wait_ge(cp_sem, t - PSUM_BANKS + 1)
                tensor.matmul(
                    out=psum[t % PSUM_BANKS][:],
                    lhsT=s_sb[w % N_S][:, lc * P : (lc + 1) * P],
                    rhs=msgs_sb[w % N_MSGS][
                        :, lc * 2 * D + b * D : lc * 2 * D + b * D + D
                    ],
                    start=start,
                    stop=stop,
                    skip_group_check=True,
                ).then_inc(mm_sem, 1)

        @block.scalar
        def _(scalar: bass.BassEngine):
            for t in range(N_TILES):
                scalar.wait_ge(mm_sem, tile_last_chunk[t] + 1)
                scalar.copy(
                    out=acc_sb[:, t * D : (t + 1) * D],
                    in_=psum[t % PSUM_BANKS][:],
                ).then_inc(cp_sem, 1)

    ctx.close()
    from concourse.library_overlay import lower_extended_insts

    lower_extended_insts(nc)
    return nc


def kernel(x, edge_index):
    x = np.asarray(x, dtype=np.float32)
    edge_index = np.asarray(edge_index)
    assert x.shape == (N_NODES, D)
    assert edge_index.shape[0] == 2

    xp_maps, slot_maps, idx_maps, dstrel_maps, iota, meta = prepare(x, edge_index)
    nc = build_program(meta)

    in_maps = [
        {
            "x": np.ascontiguousarray(xp_maps[k]),
            "idx": idx_maps[k],
            "dstrel": dstrel_maps[k],
            "iota": iota,
        }
        for k in range(N_CORES)
    ]
    import os

    trace = bool(int(os.environ.get("KERNEL_TRACE", "0")))
    res = run_bass_kernel_spmd(nc, in_maps, list(range(N_CORES)), trace=trace)
    if trace:
        kernel.last_results = res

    out = np.empty((N_NODES, D), dtype=np.float32)
    for k in range(N_CORES):
        # y rows are in permuted (tile, slot) order; slot_maps inverts it
        out[k * NODES_PER_CORE : (k + 1) * NODES_PER_CORE] = res.results[k]["y"][
            slot_maps[k]
        ]
    return out


# revision 17
# speedup vs baseline: 1.0460x; 1.0460x over previous
"""GNN message passing (gather + segment-sum) on 8 Trainium2 cores.

out[n, :] = sum over edges e with dst_e == n of x[src_e, :]

Strategy: shard edges by destination-node range (6250 nodes per core), so each
core owns a disjoint slice of the output and no cross-core reduction is
needed. x is cast to bf16 on host and packed as node PAIRS (one 256-byte
gather element = nodes 2k and 2k+1), so the indexed DMA gather needs no
f32->bf16 cast on device and all pair indices fit int16. Edges are grouped by
(dst tile, src parity); each 128-edge chunk multiplies a one-hot S matrix
(built on the vector engine from an iota compare) against the gathered bf16
pair rows, accumulating into a per-tile PSUM bank.

Gather descriptor generation on the GPSIMD Q7 cores is the bottleneck
(~8.5ns/index on one SWDGE queue). The chunk stream is cut into ~27-chunk
windows issued round-robin over all 4 SWDGE queues (4 independent Q7 core
pairs), with a tiny primer call first and back-pressure waits placed 7
windows behind so the pool engine's pipeline never drains — descriptor
generation then runs 4-wide continuously. Per-queue A/B completion
semaphores alternate every 4 windows so every wait threshold equals its
maximum possible value (no cross-DMA increment aliasing).
"""

import numpy as np

from concourse import bass, library_config, mybir
from concourse.bass_utils import run_bass_kernel_spmd

N_NODES = 50000
D = 64
N_CORES = 8
NODES_PER_CORE = N_NODES // N_CORES  # 6250
P = 128
N_TILES = (NODES_PER_CORE + P - 1) // P  # 49
N_PAIRS = N_NODES // 2  # 25000 (< 32768, fits int16)
PSUM_BANKS = 8
N_MSGS = 12  # msgs banks; gather pacing waits reference 3 rounds back
N_S = 4  # S-matrix banks

_f32 = mybir.dt.float32
_i16 = mybir.dt.int16
_bf16 = mybir.dt.bfloat16


def _round_up(a, b):
    return (a + b - 1) // b * b


N_ABS = 3  # absorber tiles (46..48) soak up edge-count variance
CAP_TARGET = 2016  # capped-tile edge budget: sides stay under 1024 after split


def _balance_tiles(dst_in_core):
    """Assign each of the core's 6250 dst nodes a (tile, slot). Tiles
    0..45 are balanced to ~CAP_TARGET edges each (so each side of the
    parity split stays under 1024 = 8 chunks); the highest-degree nodes are
    pre-routed to the 3 absorber tiles until the remaining total fits the
    capped budget. Returns slot_of_node [6250]."""
    deg = np.bincount(dst_in_core, minlength=NODES_PER_CORE)
    order = np.argsort(-deg, kind="stable")
    n_capped = N_TILES - N_ABS
    cap = np.full(N_TILES, P, dtype=np.int64)
    cap[N_TILES - 1] = NODES_PER_CORE - (N_TILES - 1) * P  # 106
    load = np.zeros(N_TILES, dtype=np.int64)
    fill = np.zeros(N_TILES, dtype=np.int64)
    slot = np.empty(NODES_PER_CORE, dtype=np.int64)

    overflow = int(deg.sum()) - n_capped * CAP_TARGET
    absorbers = list(range(n_capped, N_TILES))
    i = 0
    while overflow > 0 and i < len(order):
        n = order[i]
        open_a = [t for t in absorbers if fill[t] < cap[t]]
        if not open_a:
            break
        t = min(open_a, key=lambda t: load[t])
        slot[n] = t * P + fill[t]
        fill[t] += 1
        load[t] += deg[n]
        overflow -= int(deg[n])
        i += 1
    for n in order[i:]:
        open_t = np.nonzero(fill[:n_capped] < cap[:n_capped])[0]
        if len(open_t):
            t = open_t[np.argmin(load[open_t])]
        else:
            t = min(
                (t for t in absorbers if fill[t] < cap[t]),
                key=lambda t: load[t],
            )
        slot[n] = t * P + fill[t]
        fill[t] += 1
        load[t] += deg[n]
    return slot


def _balance_sides(src, tile_of_edge):
    """Choose a side (0/1) per source node so that every tile's edges split
    near-evenly between sides, with exactly N_PAIRS nodes per side. Batched
    greedy on the running per-tile imbalance."""
    order = np.argsort(src, kind="stable")
    s_sorted = src[order]
    t_sorted = tile_of_edge[order]
    starts = np.searchsorted(s_sorted, np.arange(N_NODES + 1))
    Dt = np.zeros(N_TILES, dtype=np.int64)
    side = np.zeros(N_NODES, dtype=np.int8)
    quota = [N_PAIRS, N_PAIRS]
    for n in range(N_NODES):
        ts = t_sorted[starts[n] : starts[n + 1]]
        if len(ts) == 0:
            s = 0 if quota[0] >= quota[1] else 1
            if quota[s] == 0:
                s = 1 - s
        else:
            s = 0 if Dt[ts].sum() < 0 else 1
            if quota[s] == 0:
                s = 1 - s
            np.add.at(Dt, ts, 1 if s == 0 else -1)
        side[n] = s
        quota[s] -= 1
    return side


def prepare(x, edge_index):
    """Host-side sharding: shard edges by dst-node range (output ownership),
    then per core (1) permute the core's dst nodes across tiles to equalize
    per-tile edge counts, (2) split source nodes into two balanced sides and
    pair one L node with one R node per 256-byte gather element, so the
    per-(tile, side) group capacities — and with them the padded gather index
    count — are minimized. Builds per-core x-pair / index / relative-dst maps."""
    import ml_dtypes

    dst = np.asarray(edge_index[0], dtype=np.int64)
    src = np.asarray(edge_index[1], dtype=np.int64)

    core = dst // NODES_PER_CORE
    dst_in_core = (dst - core * NODES_PER_CORE).astype(np.int64)

    n_groups = N_TILES * 2  # group id = tile*2 + side
    counts = np.zeros((N_CORES, n_groups), dtype=np.int64)
    per_core = []
    xp_maps = []
    slot_maps = []
    x_bf = x.astype(ml_dtypes.bfloat16)
    for k in range(N_CORES):
        sel = np.nonzero(core == k)[0]
        d_k = dst_in_core[sel]
        s_k = src[sel]
        slot = _balance_tiles(d_k)  # [6250] -> tile*128 + m
        tile_e = slot[d_k] // P
        m_e = slot[d_k] % P
        side = _balance_sides(s_k, tile_e)  # [50000] -> 0/1
        L = np.nonzero(side == 0)[0]
        R = np.nonzero(side == 1)[0]
        assert len(L) == N_PAIRS and len(R) == N_PAIRS
        pairpos = np.empty(N_NODES, dtype=np.int64)
        pairpos[L] = np.arange(N_PAIRS)
        pairpos[R] = np.arange(N_PAIRS)
        x_pairs = np.empty((N_PAIRS, 2 * D), dtype=ml_dtypes.bfloat16)
        x_pairs[:, :D] = x_bf[L]
        x_pairs[:, D:] = x_bf[R]
        xp_maps.append(x_pairs)
        slot_maps.append(slot)

        b_e = side[s_k].astype(np.int64)
        idx_e = pairpos[s_k].astype(np.int16)
        g = (tile_e * 2 + b_e).astype(np.int64)
        order = np.argsort(g, kind="stable")
        counts[k] = np.bincount(g, minlength=n_groups)
        per_core.append((idx_e[order], m_e[order], g[order]))

    # per-group 128-aligned capacity (same on all cores: SPMD layout)
    V = _round_up(np.maximum(counts.max(axis=0), 1), P).astype(np.int64)

    # stream order: (tile, parity); chunk windows cut across groups freely
    stream_off = np.zeros(n_groups, dtype=np.int64)
    off = 0
    for g in range(n_groups):
        stream_off[g] = off
        off += V[g]
    total_v = int(off)  # multiple of 128
    n_chunks = total_v // P
    idx_cols = total_v // 16

    idx_maps = []
    dstrel_maps = []

    for k in range(N_CORES):
        idx_e, m_e, g = per_core[k]
        gc = counts[k]
        starts = np.concatenate([[0], np.cumsum(gc)[:-1]])
        rank = np.arange(len(g)) - starts[g]
        pos = stream_off[g] + rank

        # pads: idx=0 (valid, gathers pair 0); killed by dstrel=-1 in S
        idx_flat = np.zeros(total_v, dtype=np.int16)
        dstrel_flat = np.full(total_v, -1.0, dtype=np.float32)
        idx_flat[pos] = idx_e
        dstrel_flat[pos] = m_e.astype(np.float32)

        # idx wrapped: element i -> partition i%16, column i//16, replicated
        # across the 8 groups of 16 partitions (required for all SWDGE queues)
        idx_wrapped = np.ascontiguousarray(
            np.tile(idx_flat.reshape(-1, 16).T, (8, 1))
        )  # [128, idx_cols]
        dstrel_cols = np.ascontiguousarray(
            dstrel_flat.reshape(-1, P).T.astype(ml_dtypes.bfloat16)
        )  # [128, n_chunks]
        idx_maps.append(idx_wrapped)
        dstrel_maps.append(dstrel_cols)

    iota = np.tile(
        np.arange(P, dtype=np.float32).astype(ml_dtypes.bfloat16), (P, 1)
    )  # [128,128]

    meta = dict(
        V=V,
        stream_off=stream_off,
        total_v=total_v,
        idx_cols=idx_cols,
        n_chunks=n_chunks,
    )
    return xp_maps, slot_maps, idx_maps, dstrel_maps, iota, meta


def build_program(meta):
    V = meta["V"]
    idx_cols = meta["idx_cols"]
    n_chunks = meta["n_chunks"]

    # global chunk list: (tile, parity) in (tile asc, parity asc) order
    chunk_tp = []
    for t in range(N_TILES):
        for b in (0, 1):
            for _ in range(int(V[t * 2 + b]) // P):
                chunk_tp.append((t, b))
    assert len(chunk_tp) == n_chunks

    tile_first_chunk = {}
    tile_last_chunk = {}
    for ci, (t, b) in enumerate(chunk_tp):
        if t not in tile_first_chunk:
            tile_first_chunk[t] = ci
        tile_last_chunk[t] = ci

    # windows: 9 rounds of 4 (window w runs on SWDGE queue w%4). Round 0
    # starts with a 1-chunk primer and stays small (fast pipeline fill),
    # round 8 is small (short drain tail), the 6 middle rounds carry the
    # bulk. The mid windows are sized per queue so every queue's TOTAL chunk
    # count is equal (the span ends when the slowest queue finishes; the
    # primer queue would otherwise finish ~7 chunks early and idle).
    first = [1, 8, 8, 8]
    # the very last window is tiny so the final drain-lag + matmul tail is
    # short; its queue is compensated in the mid rounds by the equalizer
    last = [12, 12, 12, 4]
    q_fixed = [first[q] + 16 + last[q] for q in range(4)]
    q_target = [n_chunks // 4 + (1 if q < n_chunks % 4 else 0) for q in range(4)]
    mid_q = [q_target[q] - q_fixed[q] for q in range(4)]  # over 6 mid rounds
    assert min(mid_q) >= 6, mid_q
    mid_sizes = []
    for r in range(6):
        for q in range(4):
            mid_sizes.append(mid_q[q] // 6 + (1 if r < mid_q[q] % 6 else 0))
    win_sizes = first + [16] * 4 + mid_sizes + last
    win_start = [0]
    for s in win_sizes:
        win_start.append(win_start[-1] + s)
    assert win_start[-1] == n_chunks
    n_windows = len(win_sizes)  # 36
    max_wc = max(win_sizes)

    # chunk -> window
    chunk_win = []
    for w, s in enumerate(win_sizes):
        chunk_win.extend([w] * s)

    # per-window completed tile range (for per-window output stores)
    tiles_done_by_win = []
    td = 0
    for w in range(n_windows):
        end_chunk = win_start[w + 1]
        while td < N_TILES and tile_last_chunk[td] < end_chunk:
            td += 1
        tiles_done_by_win.append(td)

    nc = bass.Bass(num_swdge_queues=4)
    x = nc.declare_dram_parameter("x", [N_PAIRS, 2 * D], _bf16, isOutput=False)
    idx = nc.declare_dram_parameter("idx", [P, idx_cols], _i16, isOutput=False)
    dstrel = nc.declare_dram_parameter("dstrel", [P, n_chunks], _bf16, isOutput=False)
    iota = nc.declare_dram_parameter("iota", [P, P], _bf16, isOutput=False)
    y = nc.declare_dram_parameter("y", [N_TILES * P, D], _f32, isOutput=True)

    import contextlib

    ctx = contextlib.ExitStack()
    idx_sb = ctx.enter_context(nc.sbuf_tensor("idx_sb", [P, idx_cols], _i16))
    dstrel_sb = ctx.enter_context(nc.sbuf_tensor("dstrel_sb", [P, n_chunks], _bf16))
    iota_sb = ctx.enter_context(nc.sbuf_tensor("iota_sb", [P, P], _bf16))
    acc_sb = ctx.enter_context(nc.sbuf_tensor("acc_sb", [P, N_TILES * D], _f32))
    msgs_sb = [
        ctx.enter_context(nc.sbuf_tensor(f"msgs{i}", [P, max_wc * 2 * D], _bf16))
        for i in range(N_MSGS)
    ]
    s_sb = [
        ctx.enter_context(nc.sbuf_tensor(f"s{i}", [P, max_wc * P], _bf16))
        for i in range(N_S)
    ]
    psum = [
        ctx.enter_context(nc.psum_tensor(f"ps{i}", [P, D], _f32))
        for i in range(PSUM_BANKS)
    ]

    # idx loaded per round (one DMA + semaphore per round of 4 windows)
    n_rounds = n_windows // 4
    spans = []
    for r in range(n_rounds):
        spans.append((win_start[4 * r] * P, win_start[4 * r + 4] * P))

    ixl_sems = [nc.alloc_semaphore(f"ixl{r}") for r in range(n_rounds)]
    dq_flat = [nc.alloc_semaphore(f"dq{i}") for i in range(12)]
    with (
        nc.Block() as block,
        nc.semaphore("ld_sem") as ld_sem,
        nc.semaphore("s_sem") as s_sem,
        nc.semaphore("mm_sem") as mm_sem,
        nc.semaphore("cp_sem") as cp_sem,
        nc.semaphore("st_sem") as st_sem,
    ):
        def dq_sem(w):
            return dq_flat[((w // 4) % 3) * 4 + (w % 4)]

        def dq_target(w):
            return 16 * (w // 12 + 1)

        @block.sync
        def _(sync: bass.BassEngine):
            for i, (c0, c1) in enumerate(spans):
                sync.dma_start(
                    out=idx_sb[:, c0 // 16 : c1 // 16],
                    in_=idx[:, c0 // 16 : c1 // 16],
                ).then_inc(ixl_sems[i], 16)
            sync.dma_start(out=dstrel_sb[:], in_=dstrel[:]).then_inc(ld_sem, 16)
            sync.dma_start(out=iota_sb[:], in_=iota[:]).then_inc(ld_sem, 16)
            # store each window's newly-completed output tiles
            n_stores = 0
            prev_td = 0
            for w in range(n_windows):
                td = tiles_done_by_win[w]
                if td == prev_td:
                    continue
                sync.wait_ge(cp_sem, td)
                sync.dma_start(
                    out=y[prev_td * P : td * P].rearrange("(t p) f -> p t f", p=P),
                    in_=acc_sb[:, prev_td * D : td * D].rearrange(
                        "p (t f) -> p t f", f=D
                    ),
                ).then_inc(st_sem, 16)
                n_stores += 1
                prev_td = td
            sync.wait_ge(st_sem, 16 * n_stores)

        @block.gpsimd
        def _(gpsimd: bass.BassEngine):
            # One wait cluster per ROUND of 4 windows (one window per SWDGE
            # queue), then 4 back-to-back gather dispatches: any wait placed
            # between gathers executes only after the preceding gather's Q7
            # desc-gen retires, so per-window waits would serialize the
            # queues. Waits reference 3 rounds back (12 msgs banks), which
            # also caps per-queue in-flight calls at 2 and makes the dq
            # A/B/C thresholds exact — do not loosen.
            gpsimd.load_library(library_config.mlp)
            assert n_windows % 4 == 0
            for r in range(n_windows // 4):
                ws = range(4 * r, 4 * r + 4)
                gpsimd.wait_ge(ixl_sems[r], 16)
                if r >= 3:
                    # msgs bank reuse: PE consumed round r-3's windows
                    gpsimd.wait_ge(mm_sem, win_start[4 * (r - 3) + 4])
                for w in ws:
                    c0, c1 = win_start[w], win_start[w + 1]
                    wc = c1 - c0
                    cap = wc * P
                    out_view = msgs_sb[w % N_MSGS][:, : wc * 2 * D].rearrange(
                        "p (c f) -> p c f", f=2 * D
                    )
                    gpsimd.dma_gather(
                        out_ap=out_view,
                        in_ap=x[:, :],
                        idxs_ap=idx_sb[:, c0 * P // 16 : c1 * P // 16],
                        num_idxs=cap,
                        num_idxs_reg=cap,
                        elem_size=2 * D,
                        single_packet=False,
                        queue_num=w % 4,
                    ).then_inc(dq_sem(w), 16)

        @block.vector
        def _(vector: bass.BassEngine):
            vector.wait_ge(ld_sem, 32)  # dstrel + iota loaded
            for w in range(n_windows):
                c0, c1 = win_start[w], win_start[w + 1]
                wc = c1 - c0
                if w >= N_S:
                    # S bank reuse: PE consumed window w-N_S
                    vector.wait_ge(mm_sem, win_start[w - N_S + 1])
                vector.tensor_tensor(
                    out=s_sb[w % N_S][:, : wc * P].rearrange(
                        "p (c f) -> p c f", f=P
                    ),
                    in0=dstrel_sb[:, c0:c1]
                    .rearrange("p (c o) -> p c o", o=1)
                    .to_broadcast([P, wc, P]),
                    in1=iota_sb[:, :]
                    .rearrange("p (o f) -> p o f", o=1)
                    .to_broadcast([P, wc, P]),
                    op=mybir.AluOpType.is_equal,
                ).then_inc(s_sem, 1)

        @block.tensor
        def _(tensor: bass.BassEngine):
            for ci, (t, b) in enumerate(chunk_tp):
                w = chunk_win[ci]
                lc = ci - win_start[w]
                if ci == win_start[w]:
                    tensor.wait_ge(s_sem, w + 1)
                    tensor.wait_ge(dq_sem(w), dq_target(w))
                start = ci == tile_first_chunk[t]
                stop = ci == tile_last_chunk[t]
                if start and t >= PSUM_BANKS:
                    tensor.wait_ge(cp_sem, t - PSUM_BANKS + 1)
                tensor.matmul(
                    out=psum[t % PSUM_BANKS][:],
                    lhsT=s_sb[w % N_S][:, lc * P : (lc + 1) * P],
                    rhs=msgs_sb[w % N_MSGS][
                        :, lc * 2 * D + b * D : lc * 2 * D + b * D + D
                    ],
                    start=start,
                    stop=stop,
                    skip_group_check=True,
                ).then_inc(mm_sem, 1)

        @block.scalar
        def _(scalar: bass.BassEngine):
            for t in range(N_TILES):
                scalar.wait_ge(mm_sem, tile_last_chunk[t] + 1)
                scalar.copy(
                    out=acc_sb[:, t * D : (t + 1) * D],
                    in_=psum[t % PSUM_BANKS][:],
                ).then_inc(cp_sem, 1)

    ctx.close()
    from concourse.library_overlay import lower_extended_insts

    lower_extended_insts(nc)
    return nc


def kernel(x, edge_index):
    x = np.asarray(x, dtype=np.float32)
    edge_index = np.asarray(edge_index)
    assert x.shape == (N_NODES, D)
    assert edge_index.shape[0] == 2

    xp_maps, slot_maps, idx_maps, dstrel_maps, iota, meta = prepare(x, edge_index)
    nc = build_program(meta)

    in_maps = [
        {
            "x": np.ascontiguousarray(xp_maps[k]),
            "idx": idx_maps[k],
            "dstrel": dstrel_maps[k],
            "iota": iota,
        }
        for k in range(N_CORES)
    ]
    import os

    trace = bool(int(os.environ.get("KERNEL_TRACE", "0")))
    res = run_bass_kernel_spmd(nc, in_maps, list(range(N_CORES)), trace=trace)
    if trace:
        kernel.last_results = res

    out = np.empty((N_NODES, D), dtype=np.float32)
    for k in range(N_CORES):
        # y rows are in permuted (tile, slot) order; slot_maps inverts it
        out[k * NODES_PER_CORE : (k + 1) * NODES_PER_CORE] = res.results[k]["y"][
            slot_maps[k]
        ]
    return out
